# revision 5
# baseline (speedup 1.0000x reference)
"""Trainium2 Bass kernel for nn_NeuralRenderer (image_size=256, F=640 faces).

Tile-culled rasterizer, 8-core pixel-sharded (interleaved 4-row groups so all
cores share one compiled program):

  host:   project faces, build per-face affine plane coefficients (fp16 h/l
          pair for exact-f32 evaluation on the PE), cull faces per 4x32-pixel
          tile (conservative NDC bbox test), pack per-tile face lists with
          data-dependent budgets baked into the compiled program.
  phase1: per tile, K=3 matmul -> [w0|w1|w2|d] planes in PSUM; ScalarE relu
          drain, DVE/GpSimd penalty sum, DVE key = -BIG*pen - d, DVE
          max/max_index = nearest visible face.  Tiles with equal budgets are
          packed into shared PSUM tiles (512-aligned slots) so the drain/
          penalty/key ops run once per group.
  gather: winner indices -> int16 -> DRAM round-trip into the 16-partition
          wrapped layout -> batched dma_gather of 256B face records
          (<=1024 indices per gather).
  phase2: exact f32 recompute of the reference's barycentrics/validity for
          the winner, texel row dma_gather at (face,i0,i1) granularity,
          arithmetic 6-way select over i2, shade, mask, store.

The d>0 visibility term is dropped from the phase-1 penalty: all camera-space
depths are positive (host-verified), so inside(w)=>d>0; phase 2 keeps the
exact d>0 test regardless.
"""

import numpy as np

IMG = 256
F = 640
NCORES = 8
NT = 64                 # pixel tiles per core (4 rows x 32 cols each)
CH = 32                 # phase-2 chunk, in tiles
PAD_PX = 2.0            # cull guard band in pixels
BMIN, BSTEP, BCAP = 32, 32, 512
NREC = 64               # padded record row (f32) = 256B
NTEXROW = F * 36        # texel rows at (face,i0,i1): 6*3 f32 used, padded
BIG = 1e34

_CACHE: dict = {}


# ----------------------------------------------------------------------------
# Host-side prep
# ----------------------------------------------------------------------------

def _geometry(vertices, faces, textures):
    f32 = np.float32
    v = np.asarray(vertices[0], f32)
    f = np.asarray(faces[0]).astype(np.int64)
    fv = v[f]

    n = np.cross(fv[:, 1] - fv[:, 0], fv[:, 2] - fv[:, 0]).astype(f32)
    nrm = np.linalg.norm(n, axis=-1, keepdims=True).astype(f32)
    n = (n / (nrm + f32(1e-8))).astype(f32)
    light = (f32(0.5) + f32(0.5) * np.maximum(n[:, 2], f32(0.0))).astype(f32)

    vc = (fv - np.array([0.0, 0.0, -2.0], f32)).astype(f32)
    zc = vc[..., 2].astype(f32)
    wfov = f32(np.tan(np.deg2rad(f32(45.0), dtype=f32), dtype=f32))
    xy = (vc[..., :2] / (zc[..., None] * wfov + f32(1e-8))).astype(f32)

    v0 = xy[:, 0]
    dd = (xy[:, 1] - v0).astype(f32)
    ee = (xy[:, 2] - v0).astype(f32)
    det = (dd[:, 0] * ee[:, 1] - dd[:, 1] * ee[:, 0]).astype(f32)
    det_ok = np.abs(det) > f32(1e-8)
    det_s = np.where(det_ok, det, f32(1.0)).astype(f32)

    x0 = v0[:, 0].astype(np.float64); y0 = v0[:, 1].astype(np.float64)
    d0 = dd[:, 0].astype(np.float64); d1 = dd[:, 1].astype(np.float64)
    e0 = ee[:, 0].astype(np.float64); e1 = ee[:, 1].astype(np.float64)
    ds = det_s.astype(np.float64)
    a1 = np.stack([e1, -e0, e0 * y0 - e1 * x0], -1) / ds[:, None]
    a2 = np.stack([-d1, d0, d1 * x0 - d0 * y0], -1) / ds[:, None]
    a0 = -a1 - a2
    a0[:, 2] += 1.0
    zc64 = zc.astype(np.float64)
    ad = a0 * zc64[:, 0:1] + a1 * zc64[:, 1:2] + a2 * zc64[:, 2:3]

    coefmax = np.max(np.abs(np.stack([a0, a1, a2, ad])), axis=(0, 2))
    bad = (~det_ok | ~np.isfinite(a0).all(1) | ~np.isfinite(a1).all(1)
           | ~np.isfinite(a2).all(1) | ~np.isfinite(ad).all(1)
           | (coefmax > 6e4) | (zc.min(1) <= 0))
    for a in (a0, a1, a2):
        a[bad] = np.array([0.0, 0.0, -1.0])
    ad[bad] = np.array([0.0, 0.0, 1.0])

    tex = np.tanh(np.asarray(textures[0], f32)).astype(f32)      # [F,6,6,6,3]
    texlit = (tex * light[:, None, None, None, None]).astype(f32)
    texrow = np.zeros((NTEXROW, NREC), f32)
    texrow[:, :18] = texlit.reshape(F * 36, 18)

    # NDC bboxes for culling (bad faces excluded everywhere)
    bbmin = xy.min(1); bbmax = xy.max(1)
    bbmin[bad] = 10.0; bbmax[bad] = 11.0

    return dict(v0=v0, dd=dd, ee=ee, det_s=det_s, det_ok=det_ok, zc=zc,
                a0=a0.astype(f32), a1=a1.astype(f32), a2=a2.astype(f32),
                ad=ad.astype(f32), texrow=texrow, bbmin=bbmin, bbmax=bbmax)


def _tile_rect(c, t):
    j, x = t // 8, t % 8
    r0 = 32 * j + 4 * c
    c0 = 32 * x
    return r0, r0 + 3, c0, c0 + 31


_PS = None

def _pixgrid():
    global _PS
    if _PS is None:
        _PS = ((np.arange(IMG, dtype=np.float32) + np.float32(0.5))
               / np.float32(IMG) * np.float32(2.0) - np.float32(1.0))
    return _PS


def _cull(geo):
    """Per-tile face lists per core + shared budgets."""
    ps = _pixgrid()
    pad = PAD_PX * (2.0 / IMG)
    bbmin, bbmax = geo["bbmin"], geo["bbmax"]
    lists = [[None] * NT for _ in range(NCORES)]
    budgets = np.zeros(NT, np.int64)
    for t in range(NT):
        cnt = 0
        for c in range(NCORES):
            r0, r1, c0, c1 = _tile_rect(c, t)
            ylo, yhi = -ps[r1] - pad, -ps[r0] + pad
            xlo, xhi = ps[c0] - pad, ps[c1] + pad
            m = ((bbmin[:, 0] <= xhi) & (bbmax[:, 0] >= xlo)
                 & (bbmin[:, 1] <= yhi) & (bbmax[:, 1] >= ylo))
            idx = np.nonzero(m)[0]
            lists[c][t] = idx
            cnt = max(cnt, len(idx))
        b = max(BMIN, -(-cnt // BSTEP) * BSTEP)
        if b > BCAP:
            raise NotImplementedError(f"tile {t} needs {cnt} faces > {BCAP}")
        budgets[t] = b
    return lists, budgets


def _plan(budgets):
    """Group tiles with equal budgets into shared PSUM tiles.

    Returns list of groups: (slot_width, [(tile, B), ...]) with slot offsets
    slot_width*i, all within a [128, 2048] PSUM tile.  Grouping is done
    independently per phase-2 chunk so early chunks finish early.
    """
    groups = []
    for ck in range(NT // CH):
        tiles = sorted(range(ck * CH, (ck + 1) * CH), key=lambda t: budgets[t])
        i = 0
        while i < len(tiles):
            b = budgets[tiles[i]]
            run = [t for t in tiles[i:] if budgets[t] == b]
            if b <= 128:
                cap, slot = 4, 512
            elif b <= 256:
                cap, slot = 2, 1024
            else:
                cap, slot = 1, 4 * b
            for k in range(0, len(run), cap):
                groups.append((int(slot), [(t, int(b)) for t in run[k:k + cap]]))
            i += len(run)
    return groups


def _pack_core(geo, lists, budgets, groups, c):
    """Per-core DRAM payloads for the compiled plan."""
    f32 = np.float32
    ncols = sum(slot * len(mem) for slot, mem in groups)
    faceB = np.zeros((3, ncols), f32)
    nrows = int(budgets.sum())
    frecT = np.zeros((nrows, NREC), f32)
    rowbase = np.zeros(NT, np.int64)
    base = 0
    for t in range(NT):
        rowbase[t] = base
        base += budgets[t]

    A = np.stack([geo["a0"], geo["a1"], geo["a2"], geo["ad"]])   # [4,F,3]
    col = 0
    for slot, mem in groups:
        for (t, b) in mem:
            li = lists[c][t]
            nb = len(li)
            blk = np.zeros((3, 4 * b), f32)
            for pl in range(4):
                blk[:, pl * b:pl * b + nb] = A[pl][li].T
                if nb < b:   # dummy: w=-1 plane, d=1 plane
                    blk[2, pl * b + nb:(pl + 1) * b] = -1.0 if pl < 3 else 1.0
            faceB[:, col:col + 4 * b] = blk
            rb = rowbase[t]
            fr = np.zeros((b, NREC), f32)
            fr[:, 6] = 1.0                   # det_s for pads
            if nb:
                fr[:nb, 0] = geo["v0"][li, 0]; fr[:nb, 1] = geo["v0"][li, 1]
                fr[:nb, 2] = geo["dd"][li, 0]; fr[:nb, 3] = geo["dd"][li, 1]
                fr[:nb, 4] = geo["ee"][li, 0]; fr[:nb, 5] = geo["ee"][li, 1]
                fr[:nb, 6] = geo["det_s"][li]
                fr[:nb, 7] = geo["det_ok"][li].astype(f32)
                fr[:nb, 8] = geo["zc"][li, 0]; fr[:nb, 9] = geo["zc"][li, 1]
                fr[:nb, 10] = geo["zc"][li, 2]
                fr[:nb, 11] = (li * 36).astype(f32)
            frecT[rb:rb + b] = fr
            col += slot
    assert col == ncols

    # pixel coordinates
    ps = _pixgrid()
    pxT = np.zeros((3, 128 * NT), np.float16)
    pxv = np.zeros((128, NT), f32)
    pyv = np.zeros((128, NT), f32)
    p = np.arange(128)
    for t in range(NT):
        r0, _, c0, _ = _tile_rect(c, t)
        rows = r0 + p // 32
        cols = c0 + p % 32
        px = ps[cols].astype(f32)
        py = (-ps[rows]).astype(f32)
        pxT[0, t * 128:(t + 1) * 128] = px
        pxT[1, t * 128:(t + 1) * 128] = py
        pxT[2, t * 128:(t + 1) * 128] = 1.0
        pxv[:, t] = px
        pyv[:, t] = py

    tbase = np.broadcast_to(rowbase.astype(np.uint32), (128, NT)).copy()

    faceBh = faceB.astype(np.float16)
    faceBl = (faceB - faceBh.astype(f32)).astype(np.float16)
    return dict(pxT=pxT, pxv=pxv, pyv=pyv, tbase=tbase,
                faceBh=faceBh, faceBl=faceBl, frecT=frecT), ncols, nrows


# ----------------------------------------------------------------------------
# Device program
# ----------------------------------------------------------------------------

def _build_program(groups, ncols, nrows):
    import concourse.bass as bass
    import concourse.bacc as bacc
    import concourse.mybir as mybir
    import concourse.tile as tile

    dt = mybir.dt
    Alu = mybir.AluOpType
    Act = mybir.ActivationFunctionType

    nc = bacc.Bacc(None, target_bir_lowering=False)
    pxT_d = nc.dram_tensor("pxT", [3, 128 * NT], dt.float16, kind="ExternalInput")
    pxv_d = nc.dram_tensor("pxv", [128, NT], dt.float32, kind="ExternalInput")
    pyv_d = nc.dram_tensor("pyv", [128, NT], dt.float32, kind="ExternalInput")
    tbase_d = nc.dram_tensor("tbase", [128, NT], dt.uint32, kind="ExternalInput")
    fBh_d = nc.dram_tensor("faceBh", [3, ncols], dt.float16, kind="ExternalInput")
    fBl_d = nc.dram_tensor("faceBl", [3, ncols], dt.float16, kind="ExternalInput")
    frecT_d = nc.dram_tensor("frecT", [nrows, NREC], dt.float32, kind="ExternalInput")
    texrow_d = nc.dram_tensor("texrow", [NTEXROW, NREC], dt.float32, kind="ExternalInput")
    img_d = nc.dram_tensor("img", [128, NT, 3], dt.float32, kind="ExternalOutput")
    scr1_d = nc.dram_tensor("scr1", [NT, 128], dt.int16, kind="ExternalOutput")
    scr2_d = nc.dram_tensor("scr2", [NT, 128], dt.int16, kind="ExternalOutput")

    NCHUNK = NT // CH

    with tile.TileContext(nc) as tc:
        with (
            tc.tile_pool(name="const", bufs=1) as cp,
            tc.tile_pool(name="work", bufs=3) as wp,
            tc.tile_pool(name="fB", bufs=4) as fp,
            tc.tile_pool(name="p2", bufs=2) as p2,
            tc.tile_pool(name="psA", bufs=2, space="PSUM") as ppA,
        ):
            pxT = cp.tile([3, 128 * NT], dt.float16)
            nc.sync.dma_start(pxT[:], pxT_d[:])
            pxv = cp.tile([128, NT], dt.float32)
            nc.sync.dma_start(pxv[:], pxv_d[:])
            pyv = cp.tile([128, NT], dt.float32)
            nc.sync.dma_start(pyv[:], pyv_d[:])
            tbase = cp.tile([128, NT], dt.uint32)
            nc.sync.dma_start(tbase[:], tbase_d[:])
            m8buf = cp.tile([128, NT, 8], dt.float32)
            i8buf = cp.tile([128, NT, 8], dt.uint32)
            gidx16 = cp.tile([128, NT], dt.int16)

            # ---------------- phase 1 ----------------
            col = 0
            for gi, (slot, mem) in enumerate(groups):
                G = len(mem)
                B = mem[0][1]
                span = slot * G
                fBh = fp.tile([3, 2048], dt.float16, tag="fh")
                nc.sync.dma_start(fBh[:, 0:span], fBh_d[:, col:col + span])
                fBl = fp.tile([3, 2048], dt.float16, tag="fl")
                nc.sync.dma_start(fBl[:, 0:span], fBl_d[:, col:col + span])
                P = ppA.tile([128, 2048], dt.float32, tag="ps")
                for m, (t, b) in enumerate(mem):
                    lhsT = pxT[:, t * 128:(t + 1) * 128]
                    s = m * slot
                    for k in range(0, 4 * b, 512):
                        e = min(4 * b, k + 512)
                        nc.tensor.matmul(P[:, s + k:s + e], lhsT,
                                         fBh[:, s + k:s + e],
                                         start=True, stop=False)
                        nc.tensor.matmul(P[:, s + k:s + e], lhsT,
                                         fBl[:, s + k:s + e],
                                         start=False, stop=True)
                psv = P[:, 0:span].rearrange("p (g s) -> p g s", s=slot)
                rAll = wp.tile([128, G, 3 * B], dt.bfloat16, tag="rAll")
                nc.scalar.activation(rAll[:], psv[:, :, 0:3 * B], Act.Relu,
                                     scale=-1.0)
                pen01 = wp.tile([128, G, B], dt.bfloat16, tag="pen01")
                nc.vector.tensor_tensor(pen01[:], rAll[:, :, 0:B],
                                        rAll[:, :, B:2 * B], op=Alu.add)
                pen = wp.tile([128, G, B], dt.bfloat16, tag="pen")
                nc.gpsimd.tensor_tensor(pen[:], pen01[:],
                                        rAll[:, :, 2 * B:3 * B], op=Alu.add)
                keyn = wp.tile([128, G, B], dt.float32, tag="keyn")
                nc.vector.scalar_tensor_tensor(
                    keyn[:], pen[:], -BIG, psv[:, :, 3 * B:4 * B],
                    op0=Alu.mult, op1=Alu.subtract)
                for m, (t, b) in enumerate(mem):
                    nc.vector.max(m8buf[:, t], keyn[:, m])
                    nc.vector.max_index(i8buf[:, t], m8buf[:, t], keyn[:, m])
                col += span
            assert col == ncols

            # winner index translation (all tiles at once)
            gall = cp.tile([128, NT], dt.uint32)
            nc.vector.tensor_tensor(gall[:], i8buf[:, :, 0], tbase[:], op=Alu.add)
            nc.vector.tensor_copy(gidx16[:], gall[:])

            # ---------------- gather + phase 2, per chunk ----------------
            for ck in range(NCHUNK):
                t0 = ck * CH

                # records
                nc.sync.dma_start(scr1_d[t0:t0 + CH, :].transpose([1, 0]),
                                  gidx16[:, t0:t0 + CH])
                idxw1 = p2.tile([128, CH * 8], dt.int16, tag="idxw1")
                srcw = (scr1_d[t0:t0 + CH].rearrange("t p -> (t p)")
                        .rearrange("(s b) -> b s", b=16))
                for a in range(8):
                    nc.sync.dma_start(idxw1[16 * a:16 * (a + 1), :], srcw)
                crec = p2.tile([128, CH, NREC], dt.float32, tag="crec")
                for j in range(CH * 128 // 1024):
                    nc.gpsimd.dma_gather(
                        out_ap=crec[:, 8 * j:8 * (j + 1)], in_ap=frecT_d[:],
                        idxs_ap=idxw1[:, 64 * j:64 * (j + 1)],
                        num_idxs=1024, num_idxs_reg=1024, elem_size=NREC)

                def tt(name, in0, in1, op, eng=None, dtype=dt.float32):
                    o = p2.tile([128, CH], dtype, tag=name)
                    (eng or nc.vector).tensor_tensor(o[:], in0, in1, op=op)
                    return o

                def ts(name, in0, s1, s2, op0, op1=None, dtype=dt.float32):
                    o = p2.tile([128, CH], dtype, tag=name)
                    if op1 is None:
                        nc.vector.tensor_scalar(o[:], in0, s1, None, op0=op0)
                    else:
                        nc.vector.tensor_scalar(o[:], in0, s1, s2, op0=op0, op1=op1)
                    return o

                ch = lambda k: crec[:, :, k]
                pxc = pxv[:, t0:t0 + CH]
                pyc = pyv[:, t0:t0 + CH]

                qx = tt("qx", pxc, ch(0), Alu.subtract)
                qy = tt("qy", pyc, ch(1), Alu.subtract, eng=nc.gpsimd)
                t1 = tt("t1", qx[:], ch(5), Alu.mult)
                t2 = tt("t2", qy[:], ch(4), Alu.mult, eng=nc.gpsimd)
                n1 = tt("n1", t1[:], t2[:], Alu.subtract)
                t3 = tt("t3", ch(2), qy[:], Alu.mult)
                t4 = tt("t4", ch(3), qx[:], Alu.mult, eng=nc.gpsimd)
                n2 = tt("n2", t3[:], t4[:], Alu.subtract)
                rdet = p2.tile([128, CH], dt.float32, tag="rdet")
                nc.vector.reciprocal(rdet[:], ch(6))

                bcat = p2.tile([128, 3, CH], dt.float32, tag="bcat")
                nc.vector.tensor_tensor(bcat[:, 1], n1[:], rdet[:], op=Alu.mult)
                nc.vector.tensor_tensor(bcat[:, 2], n2[:], rdet[:], op=Alu.mult)
                u = ts("u", bcat[:, 1], -1.0, 1.0, Alu.mult, Alu.add)
                nc.vector.tensor_tensor(bcat[:, 0], u[:], bcat[:, 2],
                                        op=Alu.subtract)

                s1v = tt("s1v", n1[:], ch(6), Alu.mult, eng=nc.gpsimd)
                g1 = ts("g1", s1v[:], 0.0, None, Alu.is_ge)
                s2v = tt("s2v", n2[:], ch(6), Alu.mult, eng=nc.gpsimd)
                g2 = ts("g2", s2v[:], 0.0, None, Alu.is_ge)
                g0 = ts("g0", bcat[:, 0], 0.0, None, Alu.is_ge)
                m0 = tt("m0", bcat[:, 0], ch(8), Alu.mult)
                m1 = tt("m1", bcat[:, 1], ch(9), Alu.mult, eng=nc.gpsimd)
                s01 = tt("s01", m0[:], m1[:], Alu.add)
                m2 = tt("m2", bcat[:, 2], ch(10), Alu.mult, eng=nc.gpsimd)
                dw = tt("dw", s01[:], m2[:], Alu.add)
                gd = ts("gd", dw[:], 0.0, None, Alu.is_gt)
                vm = tt("vm", g1[:], g2[:], Alu.mult, eng=nc.gpsimd)
                vmb = tt("vmb", vm[:], g0[:], Alu.mult)
                vmc = tt("vmc", vmb[:], gd[:], Alu.mult, eng=nc.gpsimd)
                vmd = tt("vmd", vmc[:], ch(7), Alu.mult)

                # floor of clip(6*b, 0, 5.5) on the stacked [128,3,CH] tile
                x = p2.tile([128, 3, CH], dt.float32, tag="fx")
                nc.vector.tensor_scalar(x[:], bcat[:], 6.0, 0.0,
                                        op0=Alu.mult, op1=Alu.max)
                xc = p2.tile([128, 3, CH], dt.float32, tag="fxc")
                nc.vector.tensor_scalar(xc[:], x[:], 5.5, None, op0=Alu.min)
                ji = p2.tile([128, 3, CH], dt.int32, tag="fji")
                nc.vector.tensor_copy(ji[:], xc[:])
                jf = p2.tile([128, 3, CH], dt.float32, tag="fjf")
                nc.vector.tensor_copy(jf[:], ji[:])
                gtc = p2.tile([128, 3, CH], dt.float32, tag="fgt")
                nc.vector.tensor_tensor(gtc[:], jf[:], xc[:], op=Alu.is_gt)
                ibc = p2.tile([128, 3, CH], dt.float32, tag="fib")
                nc.vector.tensor_tensor(ibc[:], jf[:], gtc[:], op=Alu.subtract)

                idx2 = p2.tile([128, CH], dt.float32, tag="idx2")
                nc.vector.scalar_tensor_tensor(idx2[:], ibc[:, 0], 6.0,
                                               ibc[:, 1], op0=Alu.mult,
                                               op1=Alu.add)
                idx2b = tt("idx2b", idx2[:], ch(11), Alu.add)
                idx2i = p2.tile([128, CH], dt.int32, tag="idx2i")
                nc.vector.tensor_copy(idx2i[:], idx2b[:])
                idx2s = p2.tile([128, CH], dt.int16, tag="idx2s")
                nc.vector.tensor_copy(idx2s[:], idx2i[:])

                # texel rows
                nc.sync.dma_start(scr2_d[t0:t0 + CH, :].transpose([1, 0]),
                                  idx2s[:])
                idxw2 = p2.tile([128, CH * 8], dt.int16, tag="idxw2")
                srcw2 = (scr2_d[t0:t0 + CH].rearrange("t p -> (t p)")
                         .rearrange("(s b) -> b s", b=16))
                for a in range(8):
                    nc.sync.dma_start(idxw2[16 * a:16 * (a + 1), :], srcw2)
                tex6 = p2.tile([128, CH, NREC], dt.float32, tag="tex6")
                for j in range(CH * 128 // 1024):
                    nc.gpsimd.dma_gather(
                        out_ap=tex6[:, 8 * j:8 * (j + 1)], in_ap=texrow_d[:],
                        idxs_ap=idxw2[:, 64 * j:64 * (j + 1)],
                        num_idxs=1024, num_idxs_reg=1024, elem_size=NREC)

                # 6-way i2 select
                oh = p2.tile([128, CH, 6], dt.float32, tag="oh")
                for j in range(6):
                    nc.vector.tensor_scalar(oh[:, :, j], ibc[:, 2], float(j),
                                            None, op0=Alu.is_equal)
                t18 = tex6[:, :, 0:18].rearrange("p c (j r) -> p c j r", r=3)
                prod = p2.tile([128, CH, 6, 3], dt.float32, tag="prod")
                nc.vector.tensor_tensor(
                    prod[:], t18,
                    oh[:].unsqueeze(3).broadcast_to([128, CH, 6, 3]),
                    op=Alu.mult)
                s01t = p2.tile([128, CH, 3], dt.float32, tag="s01t")
                nc.vector.tensor_tensor(s01t[:], prod[:, :, 0], prod[:, :, 1],
                                        op=Alu.add)
                s23t = p2.tile([128, CH, 3], dt.float32, tag="s23t")
                nc.gpsimd.tensor_tensor(s23t[:], prod[:, :, 2], prod[:, :, 3],
                                        op=Alu.add)
                s45t = p2.tile([128, CH, 3], dt.float32, tag="s45t")
                nc.vector.tensor_tensor(s45t[:], prod[:, :, 4], prod[:, :, 5],
                                        op=Alu.add)
                sA = p2.tile([128, CH, 3], dt.float32, tag="sA")
                nc.vector.tensor_tensor(sA[:], s01t[:], s23t[:], op=Alu.add)
                sel = p2.tile([128, CH, 3], dt.float32, tag="sel")
                nc.vector.tensor_tensor(sel[:], sA[:], s45t[:], op=Alu.add)

                res = p2.tile([128, CH, 3], dt.float32, tag="res")
                nc.vector.tensor_tensor(
                    res[:], sel[:],
                    vmd[:].unsqueeze(2).broadcast_to([128, CH, 3]),
                    op=Alu.mult)
                nc.sync.dma_start(img_d[:, t0:t0 + CH], res[:])

    nc.compile()
    return nc


def _get_program(key, groups, ncols, nrows):
    k = ("nc", key)
    if k not in _CACHE:
        _CACHE[k] = _build_program(groups, ncols, nrows)
    return _CACHE[k]


# ----------------------------------------------------------------------------
# Entry point
# ----------------------------------------------------------------------------

def _run(inputs, trace=False):
    from concourse.bass_utils import run_bass_kernel_spmd

    geo = _geometry(np.asarray(inputs["vertices"]),
                    np.asarray(inputs["faces"]),
                    np.asarray(inputs["textures"]))
    lists, budgets = _cull(geo)
    groups = _plan(budgets)
    in_maps = []
    for c in range(NCORES):
        payload, ncols, nrows = _pack_core(geo, lists, budgets, groups, c)
        payload["texrow"] = geo["texrow"]
        in_maps.append(payload)

    key = tuple(int(b) for b in budgets)
    nc = _get_program(key, groups, ncols, nrows)
    res = run_bass_kernel_spmd(nc, in_maps, list(range(NCORES)), trace=trace)

    # unshard: img_d [128, NT, 3] per core -> [1,3,256,256]
    full = np.zeros((3, IMG, IMG), np.float32)
    p = np.arange(128)
    for c in range(NCORES):
        out = np.asarray(res.results[c]["img"])          # [128, NT, 3]
        for t in range(NT):
            r0, _, c0, _ = _tile_rect(c, t)
            rows = r0 + p // 32
            cols = c0 + p % 32
            full[:, rows, cols] = out[:, t, :].T
    return full[None].astype(np.float32), res


def kernel(**inputs) -> np.ndarray:
    out, _ = _run(inputs, trace=False)
    return out


# revision 8
# speedup vs baseline: 1.5974x; 1.5974x over previous
"""Trainium2 Bass kernel for nn_NeuralRenderer (image_size=256, F=640 faces).

Tile-culled rasterizer, 8-core pixel-sharded (interleaved 4-row groups so all
cores share one compiled program):

  host:   project faces, build per-face affine plane coefficients (fp16 h/l
          pair for exact-f32 evaluation on the PE), cull faces per 4x32-pixel
          tile (conservative NDC bbox test), pack per-tile face lists with
          data-dependent budgets baked into the compiled program.
  phase1: per tile, K=3 matmul -> [w0|w1|w2|d] planes in PSUM; ScalarE relu
          drain, DVE/GpSimd penalty sum, DVE key = -BIG*pen - d, DVE
          max/max_index = nearest visible face.  Tiles with equal budgets are
          packed into shared PSUM tiles (512-aligned slots) so the drain/
          penalty/key ops run once per group.
  gather: winner indices -> int16 -> DRAM round-trip into the 16-partition
          wrapped layout -> batched dma_gather of 256B face records
          (<=1024 indices per gather).
  phase2: exact f32 recompute of the reference's barycentrics/validity for
          the winner, texel row dma_gather at (face,i0,i1) granularity,
          arithmetic 6-way select over i2, shade, mask, store.

The d>0 visibility term is dropped from the phase-1 penalty: all camera-space
depths are positive (host-verified), so inside(w)=>d>0; phase 2 keeps the
exact d>0 test regardless.
"""

import numpy as np

IMG = 256
F = 640
NCORES = 8
NT = 64                 # pixel tiles per core (4 rows x 32 cols each)
CH = 32                 # phase-2 chunk, in tiles
PAD_PX = 2.0            # cull guard band in pixels
BMIN, BSTEP, BCAP = 32, 32, 512
NREC = 64               # padded record row (f32) = 256B
NTEXROW = F * 36        # texel rows at (face,i0,i1): 6*3 f32 used, padded
BIG = 1e34

_CACHE: dict = {}


# ----------------------------------------------------------------------------
# Host-side prep
# ----------------------------------------------------------------------------

def _geometry(vertices, faces, textures):
    f32 = np.float32
    v = np.asarray(vertices[0], f32)
    f = np.asarray(faces[0]).astype(np.int64)
    fv = v[f]

    n = np.cross(fv[:, 1] - fv[:, 0], fv[:, 2] - fv[:, 0]).astype(f32)
    nrm = np.linalg.norm(n, axis=-1, keepdims=True).astype(f32)
    n = (n / (nrm + f32(1e-8))).astype(f32)
    light = (f32(0.5) + f32(0.5) * np.maximum(n[:, 2], f32(0.0))).astype(f32)

    vc = (fv - np.array([0.0, 0.0, -2.0], f32)).astype(f32)
    zc = vc[..., 2].astype(f32)
    wfov = f32(np.tan(np.deg2rad(f32(45.0), dtype=f32), dtype=f32))
    xy = (vc[..., :2] / (zc[..., None] * wfov + f32(1e-8))).astype(f32)

    v0 = xy[:, 0]
    dd = (xy[:, 1] - v0).astype(f32)
    ee = (xy[:, 2] - v0).astype(f32)
    det = (dd[:, 0] * ee[:, 1] - dd[:, 1] * ee[:, 0]).astype(f32)
    det_ok = np.abs(det) > f32(1e-8)
    det_s = np.where(det_ok, det, f32(1.0)).astype(f32)

    x0 = v0[:, 0].astype(np.float64); y0 = v0[:, 1].astype(np.float64)
    d0 = dd[:, 0].astype(np.float64); d1 = dd[:, 1].astype(np.float64)
    e0 = ee[:, 0].astype(np.float64); e1 = ee[:, 1].astype(np.float64)
    ds = det_s.astype(np.float64)
    a1 = np.stack([e1, -e0, e0 * y0 - e1 * x0], -1) / ds[:, None]
    a2 = np.stack([-d1, d0, d1 * x0 - d0 * y0], -1) / ds[:, None]
    a0 = -a1 - a2
    a0[:, 2] += 1.0
    zc64 = zc.astype(np.float64)
    ad = a0 * zc64[:, 0:1] + a1 * zc64[:, 1:2] + a2 * zc64[:, 2:3]

    coefmax = np.max(np.abs(np.stack([a0, a1, a2, ad])), axis=(0, 2))
    bad = (~det_ok | ~np.isfinite(a0).all(1) | ~np.isfinite(a1).all(1)
           | ~np.isfinite(a2).all(1) | ~np.isfinite(ad).all(1)
           | (coefmax > 6e4) | (zc.min(1) <= 0))
    for a in (a0, a1, a2):
        a[bad] = np.array([0.0, 0.0, -1.0])
    ad[bad] = np.array([0.0, 0.0, 1.0])

    tex = np.tanh(np.asarray(textures[0], f32)).astype(f32)      # [F,6,6,6,3]
    texlit = (tex * light[:, None, None, None, None]).astype(f32)
    texrow = np.zeros((NTEXROW, NREC), f32)
    texrow[:, :18] = texlit.reshape(F * 36, 18)

    # NDC bboxes for culling (bad faces excluded everywhere)
    bbmin = xy.min(1); bbmax = xy.max(1)
    bbmin[bad] = 10.0; bbmax[bad] = 11.0

    return dict(v0=v0, dd=dd, ee=ee, det_s=det_s, det_ok=det_ok, zc=zc,
                a0=a0.astype(f32), a1=a1.astype(f32), a2=a2.astype(f32),
                ad=ad.astype(f32), texrow=texrow, bbmin=bbmin, bbmax=bbmax)


def _tile_rect(c, t):
    j, x = t // 8, t % 8
    r0 = 32 * j + 4 * c
    c0 = 32 * x
    return r0, r0 + 3, c0, c0 + 31


_PS = None

def _pixgrid():
    global _PS
    if _PS is None:
        _PS = ((np.arange(IMG, dtype=np.float32) + np.float32(0.5))
               / np.float32(IMG) * np.float32(2.0) - np.float32(1.0))
    return _PS


def _cull(geo):
    """Per-tile face lists per core + shared budgets."""
    ps = _pixgrid()
    pad = PAD_PX * (2.0 / IMG)
    bbmin, bbmax = geo["bbmin"], geo["bbmax"]
    lists = [[None] * NT for _ in range(NCORES)]
    budgets = np.zeros(NT, np.int64)
    for t in range(NT):
        cnt = 0
        for c in range(NCORES):
            r0, r1, c0, c1 = _tile_rect(c, t)
            ylo, yhi = -ps[r1] - pad, -ps[r0] + pad
            xlo, xhi = ps[c0] - pad, ps[c1] + pad
            m = ((bbmin[:, 0] <= xhi) & (bbmax[:, 0] >= xlo)
                 & (bbmin[:, 1] <= yhi) & (bbmax[:, 1] >= ylo))
            idx = np.nonzero(m)[0]
            lists[c][t] = idx
            cnt = max(cnt, len(idx))
        b = max(BMIN, -(-cnt // BSTEP) * BSTEP)
        if b > BCAP:
            raise NotImplementedError(f"tile {t} needs {cnt} faces > {BCAP}")
        budgets[t] = b
    return lists, budgets


def _plan(budgets):
    """Group tiles with equal budgets into shared PSUM tiles.

    Returns list of groups: (slot_width, [(tile, B), ...]) with slot offsets
    slot_width*i, all within a [128, 2048] PSUM tile.  Grouping is done
    independently per phase-2 chunk so early chunks finish early.
    """
    groups = []
    for ck in range(NT // CH):
        tiles = sorted(range(ck * CH, (ck + 1) * CH), key=lambda t: budgets[t])
        i = 0
        while i < len(tiles):
            b = budgets[tiles[i]]
            run = [t for t in tiles[i:] if budgets[t] == b]
            if b <= 128:
                cap, slot = 4, 512
            elif b <= 256:
                cap, slot = 2, 1024
            else:
                cap, slot = 1, 4 * b
            for k in range(0, len(run), cap):
                groups.append((int(slot), [(t, int(b)) for t in run[k:k + cap]]))
            i += len(run)
    return groups


def _pack_core(geo, lists, budgets, groups, c):
    """Per-core DRAM payloads for the compiled plan."""
    f32 = np.float32
    ncols = sum(slot * len(mem) for slot, mem in groups)
    faceB = np.zeros((3, ncols), f32)
    nrows = int(budgets.sum())
    frecT = np.zeros((nrows, NREC), f32)
    rowbase = np.zeros(NT, np.int64)
    base = 0
    for t in range(NT):
        rowbase[t] = base
        base += budgets[t]

    A = np.stack([geo["a0"], geo["a1"], geo["a2"], geo["ad"]])   # [4,F,3]
    col = 0
    for slot, mem in groups:
        for (t, b) in mem:
            li = lists[c][t]
            nb = len(li)
            blk = np.zeros((3, 4 * b), f32)
            for pl in range(4):
                blk[:, pl * b:pl * b + nb] = A[pl][li].T
                if nb < b:   # dummy: w=-1 plane, d=1 plane
                    blk[2, pl * b + nb:(pl + 1) * b] = -1.0 if pl < 3 else 1.0
            faceB[:, col:col + 4 * b] = blk
            rb = rowbase[t]
            fr = np.zeros((b, NREC), f32)
            fr[:, 6] = 1.0                   # det_s for pads
            if nb:
                fr[:nb, 0] = geo["v0"][li, 0]; fr[:nb, 1] = geo["v0"][li, 1]
                fr[:nb, 2] = geo["dd"][li, 0]; fr[:nb, 3] = geo["dd"][li, 1]
                fr[:nb, 4] = geo["ee"][li, 0]; fr[:nb, 5] = geo["ee"][li, 1]
                fr[:nb, 6] = geo["det_s"][li]
                fr[:nb, 7] = geo["det_ok"][li].astype(f32)
                fr[:nb, 8] = geo["zc"][li, 0]; fr[:nb, 9] = geo["zc"][li, 1]
                fr[:nb, 10] = geo["zc"][li, 2]
                fr[:nb, 11] = (li * 36).astype(f32)
            frecT[rb:rb + b] = fr
            col += slot
    assert col == ncols

    # pixel coordinates
    ps = _pixgrid()
    pxT = np.zeros((3, 128 * NT), np.float16)
    pxv = np.zeros((128, NT), f32)
    pyv = np.zeros((128, NT), f32)
    p = np.arange(128)
    for t in range(NT):
        r0, _, c0, _ = _tile_rect(c, t)
        rows = r0 + p // 32
        cols = c0 + p % 32
        px = ps[cols].astype(f32)
        py = (-ps[rows]).astype(f32)
        pxT[0, t * 128:(t + 1) * 128] = px
        pxT[1, t * 128:(t + 1) * 128] = py
        pxT[2, t * 128:(t + 1) * 128] = 1.0
        pxv[:, t] = px
        pyv[:, t] = py

    tbase = np.broadcast_to(rowbase.astype(np.uint32), (128, NT)).copy()

    faceBh = faceB.astype(np.float16)
    faceBl = (faceB - faceBh.astype(f32)).astype(np.float16)
    return dict(pxT=pxT, pxv=pxv, pyv=pyv, tbase=tbase,
                faceBh=faceBh, faceBl=faceBl, frecT=frecT), ncols, nrows


# ----------------------------------------------------------------------------
# Device program
# ----------------------------------------------------------------------------

def _build_program(groups, ncols, nrows):
    import concourse.bass as bass
    import concourse.bacc as bacc
    import concourse.mybir as mybir
    import concourse.tile as tile
    import bass_rust

    dt = mybir.dt
    Alu = mybir.AluOpType
    Act = mybir.ActivationFunctionType

    nc = bacc.Bacc(None, target_bir_lowering=False)
    pxT_d = nc.dram_tensor("pxT", [3, 128 * NT], dt.float16, kind="ExternalInput")
    pxv_d = nc.dram_tensor("pxv", [128, NT], dt.float32, kind="ExternalInput")
    pyv_d = nc.dram_tensor("pyv", [128, NT], dt.float32, kind="ExternalInput")
    tbase_d = nc.dram_tensor("tbase", [128, NT], dt.uint32, kind="ExternalInput")
    fBh_d = nc.dram_tensor("faceBh", [3, ncols], dt.float16, kind="ExternalInput")
    fBl_d = nc.dram_tensor("faceBl", [3, ncols], dt.float16, kind="ExternalInput")
    frecT_d = nc.dram_tensor("frecT", [nrows, NREC], dt.float32, kind="ExternalInput")
    texrow_d = nc.dram_tensor("texrow", [NTEXROW, NREC], dt.float32, kind="ExternalInput")
    img_d = nc.dram_tensor("img", [128, NT, 3], dt.float32, kind="ExternalOutput")
    scr1_d = nc.dram_tensor("scr1", [NT * 128 + 128], dt.int16, kind="ExternalOutput")
    scr2_d = nc.dram_tensor("scr2", [NT * 128 + 128], dt.int16, kind="ExternalOutput")

    NCHUNK = NT // CH

    with tile.TileContext(nc) as tc:
        with (
            tc.tile_pool(name="const", bufs=1) as cp,
            tc.tile_pool(name="work", bufs=3) as wp,
            tc.tile_pool(name="fB", bufs=4) as fp,
            tc.tile_pool(name="p2", bufs=2) as p2,
            tc.tile_pool(name="psA", bufs=2, space="PSUM") as ppA,
        ):
            pxT = cp.tile([3, 128 * NT], dt.float16)
            nc.sync.dma_start(pxT[:], pxT_d[:])
            pxv = cp.tile([128, NT], dt.float32)
            nc.sync.dma_start(pxv[:], pxv_d[:])
            pyv = cp.tile([128, NT], dt.float32)
            nc.sync.dma_start(pyv[:], pyv_d[:])
            tbase = cp.tile([128, NT], dt.uint32)
            nc.sync.dma_start(tbase[:], tbase_d[:])
            m8buf = cp.tile([128, NT, 8], dt.float32)
            i8buf = cp.tile([128, NT, 8], dt.uint32)

            # ---------------- phase 1 ----------------
            col = 0
            for gi, (slot, mem) in enumerate(groups):
                G = len(mem)
                B = mem[0][1]
                span = slot * G
                fBh = fp.tile([3, 2048], dt.float16, tag="fh")
                nc.sync.dma_start(fBh[:, 0:span], fBh_d[:, col:col + span])
                fBl = fp.tile([3, 2048], dt.float16, tag="fl")
                nc.sync.dma_start(fBl[:, 0:span], fBl_d[:, col:col + span])
                P = ppA.tile([128, 2048], dt.float32, tag="ps")
                for m, (t, b) in enumerate(mem):
                    lhsT = pxT[:, t * 128:(t + 1) * 128]
                    s = m * slot
                    for k in range(0, 4 * b, 512):
                        e = min(4 * b, k + 512)
                        nc.tensor.matmul(P[:, s + k:s + e], lhsT,
                                         fBh[:, s + k:s + e],
                                         start=True, stop=False)
                        nc.tensor.matmul(P[:, s + k:s + e], lhsT,
                                         fBl[:, s + k:s + e],
                                         start=False, stop=True)
                psv = P[:, 0:span].rearrange("p (g s) -> p g s", s=slot)
                rAll = wp.tile([128, G, 3 * B], dt.bfloat16, tag="rAll")
                nc.scalar.activation(rAll[:], psv[:, :, 0:3 * B], Act.Relu,
                                     scale=-1.0)
                pen01 = wp.tile([128, G, B], dt.bfloat16, tag="pen01")
                nc.vector.tensor_tensor(pen01[:], rAll[:, :, 0:B],
                                        rAll[:, :, B:2 * B], op=Alu.add)
                pen = wp.tile([128, G, B], dt.bfloat16, tag="pen")
                nc.gpsimd.tensor_tensor(pen[:], pen01[:],
                                        rAll[:, :, 2 * B:3 * B], op=Alu.add)
                keyn = wp.tile([128, G, B], dt.float32, tag="keyn")
                nc.vector.scalar_tensor_tensor(
                    keyn[:], pen[:], -BIG, psv[:, :, 3 * B:4 * B],
                    op0=Alu.mult, op1=Alu.subtract)
                for m, (t, b) in enumerate(mem):
                    nc.vector.max(m8buf[:, t], keyn[:, m])
                    nc.vector.max_index(i8buf[:, t], m8buf[:, t], keyn[:, m])
                col += span
            assert col == ncols

            # ---------------- gather + phase 2, per chunk ----------------
            def wrap16(i16col, scr_d, ck, tagp):
                """i16col [128, CH] -> wrapped+replicated idxs [128, CH*8]."""
                padt = p2.tile([128, 128], dt.int16, tag=tagp + "p")
                nc.vector.memset(padt[:, CH:128], 0)
                nc.vector.tensor_copy(padt[:, 0:CH], i16col)
                cT = p2.tile([128, 128], dt.int16, tag=tagp + "t")
                nc.sync.dma_start_transpose(cT[:], padt[:])
                base = ck * CH * 128
                nc.sync.dma_start(
                    scr_d[base:base + CH * 128].rearrange("(t p) -> t p", p=128),
                    cT[0:CH, :])
                idxw = p2.tile([128, CH * 8], dt.int16, tag=tagp + "w")
                src_or = bass_rust.AP(scr_d[:].tensor, base,
                                      [[16, CH * 8], [1, 128]])
                nc.sync.dma_start_transpose(idxw[:], src_or)
                for a in range(1, 8):
                    nc.sync.dma_start(idxw[16 * a:16 * (a + 1), :],
                                      idxw[0:16, :])
                return idxw

            for ck in range(NCHUNK):
                t0 = ck * CH

                # records
                gallc = p2.tile([128, CH], dt.uint32, tag="gallc")
                nc.vector.tensor_tensor(gallc[:], i8buf[:, t0:t0 + CH, 0],
                                        tbase[:, t0:t0 + CH], op=Alu.add)
                g16c = p2.tile([128, CH], dt.int16, tag="g16c")
                nc.vector.tensor_copy(g16c[:], gallc[:])
                idxw1 = wrap16(g16c[:], scr1_d, ck, "w1")
                crec = p2.tile([128, CH, NREC], dt.float32, tag="crec")
                for j in range(CH * 128 // 1024):
                    nc.gpsimd.dma_gather(
                        out_ap=crec[:, 8 * j:8 * (j + 1)], in_ap=frecT_d[:],
                        idxs_ap=idxw1[:, 64 * j:64 * (j + 1)],
                        num_idxs=1024, num_idxs_reg=1024, elem_size=NREC)

                def tt(name, in0, in1, op, eng=None, dtype=dt.float32):
                    o = p2.tile([128, CH], dtype, tag=name)
                    (eng or nc.vector).tensor_tensor(o[:], in0, in1, op=op)
                    return o

                def ts(name, in0, s1, s2, op0, op1=None, dtype=dt.float32):
                    o = p2.tile([128, CH], dtype, tag=name)
                    if op1 is None:
                        nc.vector.tensor_scalar(o[:], in0, s1, None, op0=op0)
                    else:
                        nc.vector.tensor_scalar(o[:], in0, s1, s2, op0=op0, op1=op1)
                    return o

                ch = lambda k: crec[:, :, k]
                pxc = pxv[:, t0:t0 + CH]
                pyc = pyv[:, t0:t0 + CH]

                qx = tt("qx", pxc, ch(0), Alu.subtract)
                qy = tt("qy", pyc, ch(1), Alu.subtract, eng=nc.gpsimd)
                t1 = tt("t1", qx[:], ch(5), Alu.mult)
                t2 = tt("t2", qy[:], ch(4), Alu.mult, eng=nc.gpsimd)
                n1 = tt("n1", t1[:], t2[:], Alu.subtract)
                t3 = tt("t3", ch(2), qy[:], Alu.mult)
                t4 = tt("t4", ch(3), qx[:], Alu.mult, eng=nc.gpsimd)
                n2 = tt("n2", t3[:], t4[:], Alu.subtract)
                rdet = p2.tile([128, CH], dt.float32, tag="rdet")
                nc.vector.reciprocal(rdet[:], ch(6))

                bcat = p2.tile([128, 3, CH], dt.float32, tag="bcat")
                nc.vector.tensor_tensor(bcat[:, 1], n1[:], rdet[:], op=Alu.mult)
                nc.vector.tensor_tensor(bcat[:, 2], n2[:], rdet[:], op=Alu.mult)
                u = ts("u", bcat[:, 1], -1.0, 1.0, Alu.mult, Alu.add)
                nc.vector.tensor_tensor(bcat[:, 0], u[:], bcat[:, 2],
                                        op=Alu.subtract)

                s1v = tt("s1v", n1[:], ch(6), Alu.mult, eng=nc.gpsimd)
                g1 = ts("g1", s1v[:], 0.0, None, Alu.is_ge)
                s2v = tt("s2v", n2[:], ch(6), Alu.mult, eng=nc.gpsimd)
                g2 = ts("g2", s2v[:], 0.0, None, Alu.is_ge)
                g0 = ts("g0", bcat[:, 0], 0.0, None, Alu.is_ge)
                m0 = tt("m0", bcat[:, 0], ch(8), Alu.mult)
                m1 = tt("m1", bcat[:, 1], ch(9), Alu.mult, eng=nc.gpsimd)
                s01 = tt("s01", m0[:], m1[:], Alu.add)
                m2 = tt("m2", bcat[:, 2], ch(10), Alu.mult, eng=nc.gpsimd)
                dw = tt("dw", s01[:], m2[:], Alu.add)
                gd = ts("gd", dw[:], 0.0, None, Alu.is_gt)
                vm = tt("vm", g1[:], g2[:], Alu.mult, eng=nc.gpsimd)
                vmb = tt("vmb", vm[:], g0[:], Alu.mult)
                vmc = tt("vmc", vmb[:], gd[:], Alu.mult, eng=nc.gpsimd)
                vmd = tt("vmd", vmc[:], ch(7), Alu.mult)

                # floor of clip(6*b, 0, 5.5) on the stacked [128,3,CH] tile
                x = p2.tile([128, 3, CH], dt.float32, tag="fx")
                nc.vector.tensor_scalar(x[:], bcat[:], 6.0, 0.0,
                                        op0=Alu.mult, op1=Alu.max)
                xc = p2.tile([128, 3, CH], dt.float32, tag="fxc")
                nc.vector.tensor_scalar(xc[:], x[:], 5.5, None, op0=Alu.min)
                ji = p2.tile([128, 3, CH], dt.int32, tag="fji")
                nc.vector.tensor_copy(ji[:], xc[:])
                jf = p2.tile([128, 3, CH], dt.float32, tag="fjf")
                nc.vector.tensor_copy(jf[:], ji[:])
                gtc = p2.tile([128, 3, CH], dt.float32, tag="fgt")
                nc.vector.tensor_tensor(gtc[:], jf[:], xc[:], op=Alu.is_gt)
                ibc = p2.tile([128, 3, CH], dt.float32, tag="fib")
                nc.vector.tensor_tensor(ibc[:], jf[:], gtc[:], op=Alu.subtract)

                idx2 = p2.tile([128, CH], dt.float32, tag="idx2")
                nc.vector.scalar_tensor_tensor(idx2[:], ibc[:, 0], 6.0,
                                               ibc[:, 1], op0=Alu.mult,
                                               op1=Alu.add)
                idx2b = tt("idx2b", idx2[:], ch(11), Alu.add)
                idx2i = p2.tile([128, CH], dt.int32, tag="idx2i")
                nc.vector.tensor_copy(idx2i[:], idx2b[:])
                idx2s = p2.tile([128, CH], dt.int16, tag="idx2s")
                nc.vector.tensor_copy(idx2s[:], idx2i[:])

                # texel rows
                idxw2 = wrap16(idx2s[:], scr2_d, ck, "w2")
                tex6 = p2.tile([128, CH, NREC], dt.float32, tag="tex6")
                for j in range(CH * 128 // 1024):
                    nc.gpsimd.dma_gather(
                        out_ap=tex6[:, 8 * j:8 * (j + 1)], in_ap=texrow_d[:],
                        idxs_ap=idxw2[:, 64 * j:64 * (j + 1)],
                        num_idxs=1024, num_idxs_reg=1024, elem_size=NREC)

                # 6-way i2 select
                oh = p2.tile([128, CH, 6], dt.float32, tag="oh")
                for j in range(6):
                    nc.vector.tensor_scalar(oh[:, :, j], ibc[:, 2], float(j),
                                            None, op0=Alu.is_equal)
                t18 = tex6[:, :, 0:18].rearrange("p c (j r) -> p c j r", r=3)
                prod = p2.tile([128, CH, 6, 3], dt.float32, tag="prod")
                nc.vector.tensor_tensor(
                    prod[:], t18,
                    oh[:].unsqueeze(3).broadcast_to([128, CH, 6, 3]),
                    op=Alu.mult)
                s01t = p2.tile([128, CH, 3], dt.float32, tag="s01t")
                nc.vector.tensor_tensor(s01t[:], prod[:, :, 0], prod[:, :, 1],
                                        op=Alu.add)
                s23t = p2.tile([128, CH, 3], dt.float32, tag="s23t")
                nc.gpsimd.tensor_tensor(s23t[:], prod[:, :, 2], prod[:, :, 3],
                                        op=Alu.add)
                s45t = p2.tile([128, CH, 3], dt.float32, tag="s45t")
                nc.vector.tensor_tensor(s45t[:], prod[:, :, 4], prod[:, :, 5],
                                        op=Alu.add)
                sA = p2.tile([128, CH, 3], dt.float32, tag="sA")
                nc.vector.tensor_tensor(sA[:], s01t[:], s23t[:], op=Alu.add)
                sel = p2.tile([128, CH, 3], dt.float32, tag="sel")
                nc.vector.tensor_tensor(sel[:], sA[:], s45t[:], op=Alu.add)

                res = p2.tile([128, CH, 3], dt.float32, tag="res")
                nc.vector.tensor_tensor(
                    res[:], sel[:],
                    vmd[:].unsqueeze(2).broadcast_to([128, CH, 3]),
                    op=Alu.mult)
                nc.sync.dma_start(img_d[:, t0:t0 + CH], res[:])

    nc.compile()
    return nc


def _get_program(key, groups, ncols, nrows):
    k = ("nc", key)
    if k not in _CACHE:
        _CACHE[k] = _build_program(groups, ncols, nrows)
    return _CACHE[k]


# ----------------------------------------------------------------------------
# Entry point
# ----------------------------------------------------------------------------

def _run(inputs, trace=False):
    from concourse.bass_utils import run_bass_kernel_spmd

    geo = _geometry(np.asarray(inputs["vertices"]),
                    np.asarray(inputs["faces"]),
                    np.asarray(inputs["textures"]))
    lists, budgets = _cull(geo)
    groups = _plan(budgets)
    in_maps = []
    for c in range(NCORES):
        payload, ncols, nrows = _pack_core(geo, lists, budgets, groups, c)
        payload["texrow"] = geo["texrow"]
        in_maps.append(payload)

    key = tuple(int(b) for b in budgets)
    nc = _get_program(key, groups, ncols, nrows)
    res = run_bass_kernel_spmd(nc, in_maps, list(range(NCORES)), trace=trace)

    # unshard: img_d [128, NT, 3] per core -> [1,3,256,256]
    full = np.zeros((3, IMG, IMG), np.float32)
    p = np.arange(128)
    for c in range(NCORES):
        out = np.asarray(res.results[c]["img"])          # [128, NT, 3]
        for t in range(NT):
            r0, _, c0, _ = _tile_rect(c, t)
            rows = r0 + p // 32
            cols = c0 + p % 32
            full[:, rows, cols] = out[:, t, :].T
    return full[None].astype(np.float32), res


def kernel(**inputs) -> np.ndarray:
    out, _ = _run(inputs, trace=False)
    return out
